# revision 79
# baseline (speedup 1.0000x reference)
# Deformable-conv (DCNv2-style, scrambled-reshape variant) Trainium2 Bass kernel.
# Data-parallel over batch: 8 samples -> 8 NeuronCores.
#
# The kernel is bound by the 288 indirect-DMA gathers (one per kernel-point x
# 128-pixel chunk; the SWDGE descriptor generation costs ~1us fixed per call
# on the Pool engine = ~299us). Everything else is organized to hide under
# that stream: the front-end is phased so the first gather issues ~11us in,
# and the tail after the last gather is ~11us.
#
# Per-core pipeline (layouts derived + validated against the reference):
#   1. offset conv (18ch) + modulation conv (9ch), fp16 weights/image. The
#      mod conv runs on the transposed image via a transposed access pattern
#      on the same xpad tile (no second image needed). Outputs land in one
#      [41, 4096] fp16 tile (rows 0:18 offsets, 32:41 sigmoid(mod); engine
#      outputs need 32-aligned partition starts). The offsets path (conv ->
#      transpose -> selection -> row index) is emitted before the mod path in
#      every phase so gathers never wait on the mod conv.
#   2. PE transposes (fp16 matmuls vs identity slices, contracting only the
#      valid partition ranges) to pixel-major [128 pix, 32 chunk, 41].
#   3. Per kernel-point n2: 3 host-constant selection matmuls pick the
#      (source-pixel, source-channel) pair per partition; pointwise metadata
#      (DVE) produces a flat 2x2-patch row index + 4 bilinear*modulation
#      scales (fp16, corner-innermost).
#   4. Indirect-DMA gathers from a host-built patch table whose rows are
#      channel-outer/corner-inner (row f = [c0:4 corners, c1:4 corners, ...]),
#      so the scale multiply has packed fp16 innermost dims on every operand
#      (DVE 2x mode). One [128,1]-offset gather per (n2, chunk).
#   5. One DVE mul (scales) + one DVE pair-add (4 corners -> 2), then the
#      final corner reduction + transpose to channel-major happen on the PE:
#      2 PSUM-accumulated matmuls per 128-pixel chunk against identity.
#   6. Main conv = 9 fp16 matmuls accumulating per 512-pixel block, emitted
#      per-n2 so PSUM fills while later n2 groups still gather. Raw PSUM
#      blocks are copied contiguously to SBUF and stored; the fixed
#      pi2'->pixel permutation happens on host during unshard.
import sys

import numpy as np

sys.path.insert(0, "/opt/trn_rl_repo")

import concourse.bass as bass
import concourse.bacc as bacc
import concourse.mybir as mybir
from concourse import tile
from concourse.bass_utils import run_bass_kernel_spmd

F32 = mybir.dt.float32
F16 = mybir.dt.float16
I32 = mybir.dt.int32

B, C, H, W = 8, 128, 64, 64
OUT = 256
PIX = H * W            # 4096
KCH = 32               # pixel-major chunks (4096 / 128)
TROWS = 4224           # patch table rows (4096 + pad for f+65 reads)

_CACHE = {}


def _build_host_constants():
    if "sel" in _CACHE:
        return _CACHE
    p2 = np.arange(128)
    k2 = np.arange(KCH)
    sel = np.zeros((128, 9, 3, 128), np.float16)   # [p_src, n2, r, p2]
    basey = np.zeros((128, 9, KCH), np.float32)    # [p, n2, k]
    basex = np.zeros((128, 9, KCH), np.float32)
    for n2 in range(9):
        a2, e2 = n2 // 3, n2 % 3
        i2 = p2 % 64
        r = (i2 + e2) % 3
        n = 3 * r + a2                       # source kernel point per partition
        J = (64 * e2 + i2) // 3              # source col j per partition
        c_src = 64 * (p2 // 64) + J          # source partition in pixel-major
        for rr in range(3):
            m = r == rr
            sel[c_src[m], n2, rr, p2[m]] = 1.0
        a = n // 3
        e = n % 3
        # y_u = i + a + o_y ; i = j2 = 2*k2 + p2//64
        basey[:, n2, :] = (2 * k2[None, :] + (p2 // 64)[:, None]) + a[:, None]
        basex[:, n2, :] = (J + e)[:, None] * np.ones((1, KCH), np.float32)
    # offset-conv bias passes linearly through the selection matmul: the
    # selected offset channel per (partition, n2) is 3*r + n2//3 (y) and +9
    # (x), so the bias folds into the base positions on host (chy cached here,
    # applied per-call in _host_inputs)
    chy = np.zeros((128, 9), np.int32)
    for n2 in range(9):
        a2, e2 = n2 // 3, n2 % 3
        chy[:, n2] = 3 * (((p2 % 64) + e2) % 3) + a2
    _CACHE["chy"] = chy
    _CACHE["sel"] = sel
    _CACHE["basyx"] = np.ascontiguousarray(np.stack([basey, basex], axis=-1))
    _CACHE["ident16"] = np.eye(128, dtype=np.float16)
    return _CACHE


def _pad66(img):  # [C,64,64] -> [C, 66*66] zero-padded fp16
    p = np.zeros((C, 66, 66), np.float16)
    p[:, 1:65, 1:65] = img
    return p.reshape(C, 66 * 66)


def _patch_table(img):  # [C,64,64] f32 -> [TROWS, 512] fp16, channel-outer rows
    flat = np.zeros((C, TROWS + 65), np.float16)
    flat[:, :PIX] = img.reshape(C, PIX).astype(np.float16)
    f = np.arange(TROWS)
    tab = np.stack(
        [flat[:, f], flat[:, f + 1], flat[:, f + 64], flat[:, f + 65]], axis=-1
    )  # [C, TROWS, 4]
    return np.ascontiguousarray(tab.transpose(1, 0, 2)).reshape(TROWS, 512)


def _build_program():
    if "nc" in _CACHE:
        return _CACHE["nc"]
    nc = bacc.Bacc()
    d = {}
    d["xpad"] = nc.dram_tensor("xpad", [C, 66 * 66], F16, kind="ExternalInput")
    # first conv tile's image rows + offset-conv weights fused into one DMA
    d["xw0"] = nc.dram_tensor("xw0", [C, 660 + 9 * 18], F16, kind="ExternalInput")
    d["ptab"] = nc.dram_tensor("ptab", [TROWS, 512], F16, kind="ExternalInput")
    d["womb"] = nc.dram_tensor("womb", [C, 9, 18], F16, kind="ExternalInput")
    d["wmtb"] = nc.dram_tensor("wmtb", [C, 9, 9], F16, kind="ExternalInput")
    d["mb"] = nc.dram_tensor("mb", [9, 1], F32, kind="ExternalInput")
    d["selt"] = nc.dram_tensor("selt", [128, 9 * 3 * 128], F16, kind="ExternalInput")
    d["basyx"] = nc.dram_tensor("basyx", [128, 9 * KCH * 2], F32,
                                kind="ExternalInput")
    d["w2"] = nc.dram_tensor("w2", [C, 9 * 2 * 128], F16, kind="ExternalInput")
    d["id16"] = nc.dram_tensor("id16", [128, 128], F16, kind="ExternalInput")
    # raw main-conv PSUM blocks [sq*4 + hf*2 + q] = [128 out-ch, 512 pi2'];
    # the fixed pi2'->pixel permutation happens on host during unshard
    d["outr"] = nc.dram_tensor("outr", [16, 128, 512], F32, kind="ExternalOutput")

    AO = mybir.AluOpType

    with tile.TileContext(nc) as tc:
        with (
            tc.tile_pool(name="imgs", bufs=1) as imgs,
            tc.tile_pool(name="wts", bufs=1) as wts,
            tc.tile_pool(name="meta", bufs=1) as meta,
            tc.tile_pool(name="gbuf", bufs=8) as gbuf,
            tc.tile_pool(name="hbuf", bufs=2) as hbuf,
            tc.tile_pool(name="vbuf", bufs=2) as vbuf,
            tc.tile_pool(name="obuf", bufs=4) as obuf,
        ):
            # ---- load image + weights + constants (single DMAs each; order =
            #      first-use order so the conv pipeline starts ASAP)
            # loads ordered by first use on the critical path: the offsets
            # conv (xpad rows 0:10 + womb) unblocks first, big slices later
            xw0 = imgs.tile([C, 660 + 9 * 18], F16)
            nc.sync.dma_start(xw0[:], d["xw0"][:])
            womb = wts.tile([C, 9, 18], F16)
            nc.sync.dma_start(womb[:], d["womb"][:])
            # basyx before the big image slices: the first metadata op is
            # otherwise gated by its transfer queueing behind them
            basyx = wts.tile([128, 9, KCH, 2], F32)
            nc.sync.dma_start(basyx[:], d["basyx"][:])
            id16 = wts.tile([128, 128], F16)
            nc.sync.dma_start(id16[:], d["id16"][:])
            selt = wts.tile([128, 9, 3, 128], F16)
            nc.sync.dma_start(selt[:], d["selt"][:])
            xpad = imgs.tile([C, 66 * 66], F16)
            nc.sync.dma_start(xpad[:, 0:1188], d["xpad"][:, 0:1188])
            nc.sync.dma_start(xpad[:, 1188:66 * 34], d["xpad"][:, 1188:66 * 34])
            wmtb = wts.tile([C, 9, 9], F16)
            nc.sync.dma_start(wmtb[:], d["wmtb"][:])
            mb = wts.tile([9, 1], F32)
            nc.sync.dma_start(mb[:], d["mb"][:])
            nc.sync.dma_start(xpad[:, 66 * 34:], d["xpad"][:, 66 * 34:])
            w2 = wts.tile([C, 9, 2, 128], F16)
            nc.sync.dma_start(w2[:], d["w2"][:])
            # junk tiles for PE p-state warmup + Act table preload (values
            # never consumed)
            junka = wts.tile([128, 128], F16)
            junkb = wts.tile([128, 512], F16)
            junkc = wts.tile([18, 4], F16)
            nc.vector.memset(junka[:], 0.0)
            nc.vector.memset(junkb[:], 0.0)
            nc.scalar.copy(junkc[:], junka[0:18, 0:4])
            nc.scalar.activation(junkc[:], junka[0:18, 0:4],
                                 mybir.ActivationFunctionType.Sigmoid,
                                 bias=0.0, scale=1.0)

            # rows 0:18 offsets, 32:41 mod (engine outputs need 32-aligned
            # partition starts; rows 18:32 stay uninitialized and are never
            # read -- the transposes contract only 0:18 / 32:41)
            ocm = meta.tile([41, PIX], F16)
            opm = meta.tile([128, KCH, 41], F16)   # pixel-major
            scal = meta.tile([128, 9, KCH, 4], F16)
            idxt = meta.tile([128, 9, KCH], I32)

            with (
                tc.tile_pool(name="psc", bufs=2, space="PSUM") as psc,
                tc.tile_pool(name="pst1", bufs=2, space="PSUM") as pst1,
            ):
                # PE p-state warmup: junk matmuls keep the PE busy from t=0 so
                # the conv matmuls run at full clock once xpad lands
                warm = psc.tile([18, 512], F32, tag="po", name="warm")
                for _ in range(3):
                    nc.tensor.matmul(warm[:], junka[:, 0:18], junkb[:],
                                     start=True, stop=True)

                # Front-end in two phases: a small first phase (conv tiles
                # 0:2, chunks 0:8) so the first gathers start early, then the
                # rest in one pass. Each phase runs the full offsets path
                # (po conv -> trA -> sel -> idx) in pass A, then the mod path
                # (pm conv -> trB -> scales) in pass B, so gathers never wait
                # on the mod conv (which needs the whole image).
                Fall = meta.tile([128, 9, KCH, 2], F32)
                for tl_lo, tl_hi, k_lo, k_hi in ((0, 1, 0, 4), (1, 2, 4, 8),
                                                 (2, 8, 8, 32)):
                    HK = k_hi - k_lo
                    ho = k_lo
                    # pass A: offsets conv
                    for tl in range(tl_lo, tl_hi):
                        po = psc.tile([18, 512], F32, tag="po")
                        for t in range(9):
                            dy, dx = t // 3, t % 3
                            # tile 0 reads image rows + weights from the fused
                            # first-DMA tile so it unblocks ~1us earlier
                            src = xw0 if tl == 0 else xpad
                            rhs1 = bass.AP(
                                tensor=src[:].tensor,
                                offset=src[:].offset + dy * 66 + dx + tl * 8 * 66,
                                ap=[list(src[:].ap[0]), [66, 8], [1, 64]],
                            )
                            lw = bass.AP(
                                tensor=xw0[:].tensor,
                                offset=xw0[:].offset + 660 + t * 18,
                                ap=[list(xw0[:].ap[0]), [1, 18]],
                            ) if tl == 0 else womb[:, t, :]
                            nc.tensor.matmul(po[:], lw, rhs1,
                                             start=(t == 0), stop=(t == 8))
                        # bias folded into basyx on host; tile 0 converts on
                        # the (idle) DVE to shorten the first-gather chain
                        if tl < 1:
                            nc.vector.tensor_copy(
                                ocm[0:18, tl * 512:(tl + 1) * 512], po[:])
                        else:
                            nc.scalar.copy(
                                ocm[0:18, tl * 512:(tl + 1) * 512], po[:])
                    for tl in range(tl_lo, tl_hi):
                        ptA = pst1.tile([128, 4, 18], F32, tag="ptA", bufs=1)
                        for k4 in range(4):
                            k = tl * 4 + k4
                            nc.tensor.matmul(
                                ptA[:, k4, :],
                                ocm[0:18, k * 128:(k + 1) * 128],
                                id16[0:18, 0:18], start=True, stop=True)
                        dstA = bass.AP(
                            tensor=opm[:].tensor,
                            offset=opm[:].offset + tl * 4 * 41,
                            ap=[list(opm[:].ap[0]), [41, 4], [1, 18]],
                        )
                        nc.vector.tensor_copy(dstA, ptA[:])
                    # pass A metadata: -> flat row idx (+ frac, kept for B)
                    for n2 in range(9):
                        a2 = n2 // 3
                        oyx = pst1.tile([128, HK, 2], F32, tag="sel")
                        for r in range(3):
                            ch = 3 * r + a2
                            rhs = bass.AP(
                                tensor=opm[:].tensor,
                                offset=opm[:].offset + ch + ho * 41,
                                ap=[list(opm[:].ap[0]), [41, HK], [9, 2]],
                            )
                            nc.tensor.matmul(oyx[:], selt[:, n2, r, :], rhs,
                                             start=(r == 0), stop=(r == 2))
                        P = meta.tile([128, HK, 2], F32, tag="P")
                        nc.vector.tensor_add(P[:], oyx[:],
                                             basyx[:, n2, ho:ho + HK, :])
                        nc.vector.tensor_scalar(P[:], P[:], 0.0, 63.0,
                                                AO.max, AO.min)
                        R0 = meta.tile([128, HK, 2], F32, tag="R0")
                        nc.vector.tensor_scalar(R0[:], P[:], -0.5, 12582912.0,
                                                AO.add, AO.add)
                        nc.vector.tensor_scalar_add(R0[:], R0[:], -12582912.0)
                        nc.vector.scalar_tensor_tensor(
                            idxt[:, n2, ho:ho + HK], R0[:, :, 1], 64.0,
                            R0[:, :, 0], AO.mult, AO.add)
                        # frac (for scales, pass B) computed off the idx path
                        F = Fall[:, n2, ho:ho + HK, :]
                        nc.vector.tensor_sub(F, P[:], R0[:])
                    # pass B: mod conv
                    for tl in range(tl_lo, tl_hi):
                        pm = psc.tile([9, 512], F32, tag="pm")
                        for t in range(9):
                            dy, dx = t // 3, t % 3
                            rhs2 = bass.AP(
                                tensor=xpad[:].tensor,
                                offset=xpad[:].offset + dx * 66 + dy + tl * 8,
                                ap=[list(xpad[:].ap[0]), [1, 8], [66, 64]],
                            )
                            nc.tensor.matmul(pm[:], wmtb[:, t, :], rhs2,
                                             start=(t == 0), stop=(t == 8))
                        nc.scalar.activation(
                            ocm[32:41, tl * 512:(tl + 1) * 512], pm[:],
                            mybir.ActivationFunctionType.Sigmoid,
                            bias=mb[:], scale=1.0)
                    for tl in range(tl_lo, tl_hi):
                        ptB = pst1.tile([128, 4, 9], F32, tag="ptB", bufs=1)
                        for k4 in range(4):
                            k = tl * 4 + k4
                            nc.tensor.matmul(
                                ptB[:, k4, :],
                                ocm[32:41, k * 128:(k + 1) * 128],
                                id16[32:41, 32:41], start=True, stop=True)
                        dstB = bass.AP(
                            tensor=opm[:].tensor,
                            offset=opm[:].offset + tl * 4 * 41 + 32,
                            ap=[list(opm[:].ap[0]), [41, 4], [1, 9]],
                        )
                        nc.vector.tensor_copy(dstB, ptB[:])
                    # pass B metadata: bilinear*modulation scales (fp16)
                    for n2 in range(9):
                        F = Fall[:, n2, ho:ho + HK, :]
                        mrow = opm[:, ho:ho + HK, 32 + n2]
                        v1 = meta.tile([128, HK], F32, tag="v1")
                        v0 = meta.tile([128, HK], F32, tag="v0")
                        sc4 = meta.tile([128, 4, HK], F32, tag="sc4")
                        nc.vector.tensor_mul(v1[:], mrow, F[:, :, 1])
                        nc.vector.tensor_sub(v0[:], mrow, v1[:])
                        nc.vector.tensor_mul(sc4[:, 1, :], v0[:], F[:, :, 0])
                        nc.vector.tensor_sub(sc4[:, 0, :], v0[:], sc4[:, 1, :])
                        nc.vector.tensor_mul(sc4[:, 3, :], v1[:], F[:, :, 0])
                        nc.vector.tensor_sub(sc4[:, 2, :], v1[:], sc4[:, 3, :])
                        # convert to fp16 [k, corner]-interleaved in one copy
                        csrc = bass.AP(
                            tensor=sc4[:].tensor, offset=sc4[:].offset,
                            ap=[list(sc4[:].ap[0]), [1, HK], [HK, 4]],
                        )
                        cdst = bass.AP(
                            tensor=scal[:].tensor,
                            offset=scal[:].offset + n2 * (KCH * 4) + ho * 4,
                            ap=[list(scal[:].ap[0]), [4, HK], [1, 4]],
                        )
                        nc.vector.tensor_copy(cdst, csrc)

            with (
                tc.tile_pool(name="pst", bufs=3, space="PSUM") as pst,
                tc.tile_pool(name="psm", bufs=1, space="PSUM") as psm,
            ):
                # ---- per spatial-quarter: gather + scale + reduce-transpose;
                #      main-conv matmuls interleave per n2 (PSUM accumulates
                #      while later n2 groups are still gathering)
                for sq in range(4):
                    vc = vbuf.tile([C, 9, 1024], F16, tag="vc")
                    accq = {}
                    for hf in range(2):
                        for tl2 in range(2):
                            accq[hf, tl2] = psm.tile(
                                [128, 512], F32, tag=f"mm{hf}{tl2}",
                                name=f"acc{hf}{tl2}")
                    for n2 in range(9):
                        g = gbuf.tile([128, 8, 512], F16, tag="g")
                        for kk in range(8):
                            k = sq * 8 + kk
                            dstg = bass.AP(
                                tensor=g[:].tensor,
                                offset=g[:].offset + kk * 512,
                                ap=[list(g[:].ap[0]), [1, 512]],
                            )
                            nc.gpsimd.indirect_dma_start(
                                out=dstg, out_offset=None,
                                in_=d["ptab"][:],
                                in_offset=bass.IndirectOffsetOnAxis(
                                    ap=idxt[:, n2, k:k + 1], axis=0),
                            )
                        h = hbuf.tile([128, 8, 128, 2], F16, tag="h")
                        # the very last group runs in 2-chunk units to halve
                        # the end-of-kernel dependency chain
                        last = (sq == 3 and n2 == 8)
                        units = ((0, 2), (2, 4), (4, 6), (6, 8)) if last \
                            else ((0, 4), (4, 8))
                        for klo, khi in units:
                            w = khi - klo
                            gv = bass.AP(
                                tensor=g[:].tensor,
                                offset=g[:].offset + klo * 512,
                                ap=[list(g[:].ap[0]), [512, w], [4, 128], [1, 4]],
                            )
                            sv = bass.AP(
                                tensor=scal[:].tensor,
                                offset=(scal[:].offset + n2 * (KCH * 4)
                                        + sq * 32 + klo * 4),
                                ap=[list(scal[:].ap[0]), [4, w], [0, 128], [1, 4]],
                            )
                            nc.vector.tensor_mul(gv, gv, sv)
                            ha = bass.AP(
                                tensor=g[:].tensor,
                                offset=g[:].offset + klo * 512,
                                ap=[list(g[:].ap[0]), [512, w], [4, 128], [1, 2]],
                            )
                            hb = bass.AP(
                                tensor=g[:].tensor,
                                offset=g[:].offset + klo * 512 + 2,
                                ap=[list(g[:].ap[0]), [512, w], [4, 128], [1, 2]],
                            )
                            hd = bass.AP(
                                tensor=h[:].tensor,
                                offset=h[:].offset + klo * 256,
                                ap=[list(h[:].ap[0]), [256, w], [2, 128], [1, 2]],
                            )
                            nc.vector.tensor_add(hd, ha, hb)
                            acc = pst.tile([128, w * 128], F32,
                                           tag="trs" if last else "tr",
                                           bufs=1 if last else None)
                            for kki in range(w):
                                kk = klo + kki
                                for j in range(2):
                                    lhsT = bass.AP(
                                        tensor=h[:].tensor,
                                        offset=h[:].offset + kk * 256 + j,
                                        ap=[list(h[:].ap[0]), [2, 128]],
                                    )
                                    nc.tensor.matmul(
                                        acc[:, kki * 128:(kki + 1) * 128],
                                        lhsT, id16[:],
                                        start=(j == 0), stop=(j == 1))
                            nc.scalar.copy(
                                vc[:, n2, klo * 128:khi * 128], acc[:])
                            q = klo // 4
                            co = (klo - q * 4) * 128
                            # close accq(1,*) first at the very end so its
                            # Act copy overlaps accq(0,*)'s final matmul
                            for hf in ((1, 0) if (last and klo == 6)
                                       else (0, 1)):
                                nc.tensor.matmul(
                                    accq[hf, q][:, co:co + w * 128],
                                    w2[:, n2, hf, :],
                                    vc[:, n2, klo * 128:khi * 128],
                                    start=(n2 == 0), stop=(n2 == 8),
                                    skip_group_check=last)

                    # store raw blocks (contiguous; host unscrambles); the two
                    # late q=1 copies run in parallel on Act and DVE
                    for hf in range(2):
                        for q in range(2):
                            outq = obuf.tile([128, 512], F32, tag="oq",
                                             name="outq")
                            if q == 1 and hf == 0:
                                nc.vector.tensor_copy(outq[:], accq[hf, q][:])
                            else:
                                nc.scalar.copy(outq[:], accq[hf, q][:])
                            nc.sync.dma_start(
                                d["outr"][sq * 4 + hf * 2 + q], outq[:])

    nc.compile()
    _CACHE["nc"] = nc
    return nc


def _host_inputs(b_x, offset_w, offset_b, mod_w, mod_b, conv_w):
    hc = _build_host_constants()
    img = b_x.astype(np.float32)
    womb = np.zeros((C, 9, 18), np.float16)
    wmtb = np.zeros((C, 9, 9), np.float16)
    for t in range(9):
        dy, dx = t // 3, t % 3
        womb[:, t, :] = offset_w[:, :, dy, dx].T
        wmtb[:, 3 * dx + dy, :] = mod_w[:, :, dy, dx].T
    w2 = np.zeros((C, 9, 2, 128), np.float16)
    for n2 in range(9):
        a2, e2 = n2 // 3, n2 % 3
        for hf in range(2):
            w2[:, n2, hf, :] = conv_w[128 * hf:128 * (hf + 1), :, a2, e2].T
    # fold the offset-conv bias into the base positions (it passes linearly
    # through the selection matmul; selected channel = chy (y) / chy+9 (x))
    ob32 = offset_b.astype(np.float32)
    basyx = hc["basyx"].copy()
    basyx[:, :, :, 0] += ob32[hc["chy"]][:, :, None]
    basyx[:, :, :, 1] += ob32[hc["chy"] + 9][:, :, None]
    xpad = _pad66(img)
    return {
        "xpad": xpad,
        "xw0": np.ascontiguousarray(
            np.concatenate([xpad[:, 0:660], womb.reshape(C, 162)], axis=1)),
        "ptab": _patch_table(img),
        "womb": womb,
        "wmtb": wmtb,
        "mb": mod_b.reshape(9, 1).astype(np.float32),
        "selt": hc["sel"].reshape(128, 9 * 3 * 128),
        "basyx": basyx.reshape(128, 9 * KCH * 2),
        "w2": w2.reshape(C, 9 * 2 * 128),
        "id16": hc["ident16"],
    }


def kernel(x, offset_w, offset_b, mod_w, mod_b, conv_w):
    nc = _build_program()
    in_maps = [
        _host_inputs(x[b], offset_w, offset_b, mod_w, mod_b, conv_w)
        for b in range(B)
    ]
    res = run_bass_kernel_spmd(nc, in_maps, core_ids=list(range(B)))
    out = np.empty((B, OUT, H, W), np.float32)
    for b in range(B):
        # outr[sq*4 + hf*2 + q] = [128 o, 512 pi2'] with
        # pi2' = (2sq+q)*512 + q2, j2 = 8*(2sq+q) + q2//64, i2 = q2%64
        outr = res.results[b]["outr"].reshape(4, 2, 2, 128, 8, 64)
        for sq in range(4):
            for hf in range(2):
                for q in range(2):
                    j2 = 16 * sq + 8 * q
                    out[b, 128 * hf:128 * (hf + 1), :, j2:j2 + 8] = (
                        outr[sq, hf, q].transpose(0, 2, 1))
    return out


if __name__ == "__main__":
    rng = np.random.default_rng(0)
    ins = {
        "x": rng.standard_normal((B, C, H, W), dtype=np.float32),
        "offset_w": (rng.standard_normal((18, C, 3, 3)) / 34).astype(np.float32),
        "offset_b": (rng.standard_normal(18) * 0.01).astype(np.float32),
        "mod_w": (rng.standard_normal((9, C, 3, 3)) / 34).astype(np.float32),
        "mod_b": (rng.standard_normal(9) * 0.01).astype(np.float32),
        "conv_w": (rng.standard_normal((OUT, C, 3, 3)) / 34).astype(np.float32),
    }
    o = kernel(**ins)
    print("out", o.shape, o.dtype, np.abs(o).max())


# revision 80
# speedup vs baseline: 1.0019x; 1.0019x over previous
# Deformable-conv (DCNv2-style, scrambled-reshape variant) Trainium2 Bass kernel.
# Data-parallel over batch: 8 samples -> 8 NeuronCores.
#
# The kernel is bound by the 288 indirect-DMA gathers (one per kernel-point x
# 128-pixel chunk; the SWDGE descriptor generation costs ~1us fixed per call
# on the Pool engine = ~299us). Everything else is organized to hide under
# that stream: the front-end is phased so the first gather issues ~11us in,
# and the tail after the last gather is ~11us.
#
# Per-core pipeline (layouts derived + validated against the reference):
#   1. offset conv (18ch) + modulation conv (9ch), fp16 weights/image. The
#      mod conv runs on the transposed image via a transposed access pattern
#      on the same xpad tile (no second image needed). Outputs land in one
#      [41, 4096] fp16 tile (rows 0:18 offsets, 32:41 sigmoid(mod); engine
#      outputs need 32-aligned partition starts). The offsets path (conv ->
#      transpose -> selection -> row index) is emitted before the mod path in
#      every phase so gathers never wait on the mod conv.
#   2. PE transposes (fp16 matmuls vs identity slices, contracting only the
#      valid partition ranges) to pixel-major [128 pix, 32 chunk, 41].
#   3. Per kernel-point n2: 3 host-constant selection matmuls pick the
#      (source-pixel, source-channel) pair per partition; pointwise metadata
#      (DVE) produces a flat 2x2-patch row index + 4 bilinear*modulation
#      scales (fp16, corner-innermost).
#   4. Indirect-DMA gathers from a host-built patch table whose rows are
#      channel-outer/corner-inner (row f = [c0:4 corners, c1:4 corners, ...]),
#      so the scale multiply has packed fp16 innermost dims on every operand
#      (DVE 2x mode). One [128,1]-offset gather per (n2, chunk).
#   5. One DVE mul (scales) + one DVE pair-add (4 corners -> 2), then the
#      final corner reduction + transpose to channel-major happen on the PE:
#      2 PSUM-accumulated matmuls per 128-pixel chunk against identity.
#   6. Main conv = 9 fp16 matmuls accumulating per 512-pixel block, emitted
#      per-n2 so PSUM fills while later n2 groups still gather. Raw PSUM
#      blocks are copied contiguously to SBUF and stored; the fixed
#      pi2'->pixel permutation happens on host during unshard.
import sys

import numpy as np

sys.path.insert(0, "/opt/trn_rl_repo")

import concourse.bass as bass
import concourse.bacc as bacc
import concourse.mybir as mybir
from concourse import tile
from concourse.bass_utils import run_bass_kernel_spmd

F32 = mybir.dt.float32
F16 = mybir.dt.float16
I32 = mybir.dt.int32

B, C, H, W = 8, 128, 64, 64
OUT = 256
PIX = H * W            # 4096
KCH = 32               # pixel-major chunks (4096 / 128)
TROWS = 4224           # patch table rows (4096 + pad for f+65 reads)

_CACHE = {}


def _build_host_constants():
    if "sel" in _CACHE:
        return _CACHE
    p2 = np.arange(128)
    k2 = np.arange(KCH)
    sel = np.zeros((128, 9, 3, 128), np.float16)   # [p_src, n2, r, p2]
    basey = np.zeros((128, 9, KCH), np.float32)    # [p, n2, k]
    basex = np.zeros((128, 9, KCH), np.float32)
    for n2 in range(9):
        a2, e2 = n2 // 3, n2 % 3
        i2 = p2 % 64
        r = (i2 + e2) % 3
        n = 3 * r + a2                       # source kernel point per partition
        J = (64 * e2 + i2) // 3              # source col j per partition
        c_src = 64 * (p2 // 64) + J          # source partition in pixel-major
        for rr in range(3):
            m = r == rr
            sel[c_src[m], n2, rr, p2[m]] = 1.0
        a = n // 3
        e = n % 3
        # y_u = i + a + o_y ; i = j2 = 2*k2 + p2//64
        basey[:, n2, :] = (2 * k2[None, :] + (p2 // 64)[:, None]) + a[:, None]
        basex[:, n2, :] = (J + e)[:, None] * np.ones((1, KCH), np.float32)
    # offset-conv bias passes linearly through the selection matmul: the
    # selected offset channel per (partition, n2) is 3*r + n2//3 (y) and +9
    # (x), so the bias folds into the base positions on host (chy cached here,
    # applied per-call in _host_inputs)
    chy = np.zeros((128, 9), np.int32)
    for n2 in range(9):
        a2, e2 = n2 // 3, n2 % 3
        chy[:, n2] = 3 * (((p2 % 64) + e2) % 3) + a2
    _CACHE["chy"] = chy
    _CACHE["sel"] = sel
    _CACHE["basyx"] = np.ascontiguousarray(np.stack([basey, basex], axis=-1))
    _CACHE["ident16"] = np.eye(128, dtype=np.float16)
    return _CACHE


def _pad66(img):  # [C,64,64] -> [C, 66*66] zero-padded fp16
    p = np.zeros((C, 66, 66), np.float16)
    p[:, 1:65, 1:65] = img
    return p.reshape(C, 66 * 66)


def _patch_table(img):  # [C,64,64] f32 -> [TROWS, 512] fp16, channel-outer rows
    flat = np.zeros((C, TROWS + 65), np.float16)
    flat[:, :PIX] = img.reshape(C, PIX).astype(np.float16)
    f = np.arange(TROWS)
    tab = np.stack(
        [flat[:, f], flat[:, f + 1], flat[:, f + 64], flat[:, f + 65]], axis=-1
    )  # [C, TROWS, 4]
    return np.ascontiguousarray(tab.transpose(1, 0, 2)).reshape(TROWS, 512)


def _build_program():
    if "nc" in _CACHE:
        return _CACHE["nc"]
    nc = bacc.Bacc()
    d = {}
    d["xpad"] = nc.dram_tensor("xpad", [C, 66 * 66], F16, kind="ExternalInput")
    # first conv tile's image rows + offset-conv weights fused into one DMA
    d["xw0"] = nc.dram_tensor("xw0", [C, 660 + 9 * 18], F16, kind="ExternalInput")
    d["ptab"] = nc.dram_tensor("ptab", [TROWS, 512], F16, kind="ExternalInput")
    d["womb"] = nc.dram_tensor("womb", [C, 9, 18], F16, kind="ExternalInput")
    d["wmtb"] = nc.dram_tensor("wmtb", [C, 9, 9], F16, kind="ExternalInput")
    d["mb"] = nc.dram_tensor("mb", [9, 1], F32, kind="ExternalInput")
    d["selt"] = nc.dram_tensor("selt", [128, 9 * 3 * 128], F16, kind="ExternalInput")
    d["basyx"] = nc.dram_tensor("basyx", [128, 9 * KCH * 2], F32,
                                kind="ExternalInput")
    d["w2"] = nc.dram_tensor("w2", [C, 9 * 2 * 128], F16, kind="ExternalInput")
    d["id16"] = nc.dram_tensor("id16", [128, 128], F16, kind="ExternalInput")
    # raw main-conv PSUM blocks [sq*4 + hf*2 + q] = [128 out-ch, 512 pi2'];
    # the fixed pi2'->pixel permutation happens on host during unshard
    d["outr"] = nc.dram_tensor("outr", [16, 128, 512], F32, kind="ExternalOutput")

    AO = mybir.AluOpType

    with tile.TileContext(nc) as tc:
        with (
            tc.tile_pool(name="imgs", bufs=1) as imgs,
            tc.tile_pool(name="wts", bufs=1) as wts,
            tc.tile_pool(name="meta", bufs=1) as meta,
            tc.tile_pool(name="gbuf", bufs=8) as gbuf,
            tc.tile_pool(name="hbuf", bufs=2) as hbuf,
            tc.tile_pool(name="vbuf", bufs=2) as vbuf,
            tc.tile_pool(name="obuf", bufs=4) as obuf,
        ):
            # ---- load image + weights + constants (single DMAs each; order =
            #      first-use order so the conv pipeline starts ASAP)
            # loads ordered by first use on the critical path: the offsets
            # conv (xpad rows 0:10 + womb) unblocks first, big slices later
            xw0 = imgs.tile([C, 660 + 9 * 18], F16)
            nc.sync.dma_start(xw0[:], d["xw0"][:])
            # metadata-path constants (selection matrices, base positions,
            # identity) ahead of everything else: the first-gather chain is
            # bound by their arrival, not by the later conv tiles' inputs
            id16 = wts.tile([128, 128], F16)
            nc.sync.dma_start(id16[:], d["id16"][:])
            selt = wts.tile([128, 9, 3, 128], F16)
            nc.sync.dma_start(selt[:], d["selt"][:])
            basyx = wts.tile([128, 9, KCH, 2], F32)
            nc.sync.dma_start(basyx[:], d["basyx"][:])
            womb = wts.tile([C, 9, 18], F16)
            nc.sync.dma_start(womb[:], d["womb"][:])
            xpad = imgs.tile([C, 66 * 66], F16)
            nc.sync.dma_start(xpad[:, 0:1188], d["xpad"][:, 0:1188])
            nc.sync.dma_start(xpad[:, 1188:66 * 34], d["xpad"][:, 1188:66 * 34])
            wmtb = wts.tile([C, 9, 9], F16)
            nc.sync.dma_start(wmtb[:], d["wmtb"][:])
            mb = wts.tile([9, 1], F32)
            nc.sync.dma_start(mb[:], d["mb"][:])
            nc.sync.dma_start(xpad[:, 66 * 34:], d["xpad"][:, 66 * 34:])
            w2 = wts.tile([C, 9, 2, 128], F16)
            nc.sync.dma_start(w2[:], d["w2"][:])
            # junk tiles for PE p-state warmup + Act table preload (values
            # never consumed)
            junka = wts.tile([128, 128], F16)
            junkb = wts.tile([128, 512], F16)
            junkc = wts.tile([18, 4], F16)
            nc.vector.memset(junka[:], 0.0)
            nc.vector.memset(junkb[:], 0.0)
            nc.scalar.copy(junkc[:], junka[0:18, 0:4])
            nc.scalar.activation(junkc[:], junka[0:18, 0:4],
                                 mybir.ActivationFunctionType.Sigmoid,
                                 bias=0.0, scale=1.0)

            # rows 0:18 offsets, 32:41 mod (engine outputs need 32-aligned
            # partition starts; rows 18:32 stay uninitialized and are never
            # read -- the transposes contract only 0:18 / 32:41)
            ocm = meta.tile([41, PIX], F16)
            opm = meta.tile([128, KCH, 41], F16)   # pixel-major
            scal = meta.tile([128, 9, KCH, 4], F16)
            idxt = meta.tile([128, 9, KCH], I32)

            with (
                tc.tile_pool(name="psc", bufs=2, space="PSUM") as psc,
                tc.tile_pool(name="pst1", bufs=2, space="PSUM") as pst1,
            ):
                # PE p-state warmup: junk matmuls keep the PE busy from t=0 so
                # the conv matmuls run at full clock once xpad lands
                warm = psc.tile([18, 512], F32, tag="po", name="warm")
                for _ in range(3):
                    nc.tensor.matmul(warm[:], junka[:, 0:18], junkb[:],
                                     start=True, stop=True)

                # Front-end in two phases: a small first phase (conv tiles
                # 0:2, chunks 0:8) so the first gathers start early, then the
                # rest in one pass. Each phase runs the full offsets path
                # (po conv -> trA -> sel -> idx) in pass A, then the mod path
                # (pm conv -> trB -> scales) in pass B, so gathers never wait
                # on the mod conv (which needs the whole image).
                Fall = meta.tile([128, 9, KCH, 2], F32)
                for tl_lo, tl_hi, k_lo, k_hi in ((0, 1, 0, 4), (1, 2, 4, 8),
                                                 (2, 8, 8, 32)):
                    HK = k_hi - k_lo
                    ho = k_lo
                    # pass A: offsets conv
                    for tl in range(tl_lo, tl_hi):
                        po = psc.tile([18, 512], F32, tag="po")
                        for t in range(9):
                            dy, dx = t // 3, t % 3
                            # tile 0 reads image rows + weights from the fused
                            # first-DMA tile so it unblocks ~1us earlier
                            src = xw0 if tl == 0 else xpad
                            rhs1 = bass.AP(
                                tensor=src[:].tensor,
                                offset=src[:].offset + dy * 66 + dx + tl * 8 * 66,
                                ap=[list(src[:].ap[0]), [66, 8], [1, 64]],
                            )
                            lw = bass.AP(
                                tensor=xw0[:].tensor,
                                offset=xw0[:].offset + 660 + t * 18,
                                ap=[list(xw0[:].ap[0]), [1, 18]],
                            ) if tl == 0 else womb[:, t, :]
                            nc.tensor.matmul(po[:], lw, rhs1,
                                             start=(t == 0), stop=(t == 8))
                        # bias folded into basyx on host; tile 0 converts on
                        # the (idle) DVE to shorten the first-gather chain
                        if tl < 1:
                            nc.vector.tensor_copy(
                                ocm[0:18, tl * 512:(tl + 1) * 512], po[:])
                        else:
                            nc.scalar.copy(
                                ocm[0:18, tl * 512:(tl + 1) * 512], po[:])
                    for tl in range(tl_lo, tl_hi):
                        ptA = pst1.tile([128, 4, 18], F32, tag="ptA", bufs=1)
                        for k4 in range(4):
                            k = tl * 4 + k4
                            nc.tensor.matmul(
                                ptA[:, k4, :],
                                ocm[0:18, k * 128:(k + 1) * 128],
                                id16[0:18, 0:18], start=True, stop=True)
                        dstA = bass.AP(
                            tensor=opm[:].tensor,
                            offset=opm[:].offset + tl * 4 * 41,
                            ap=[list(opm[:].ap[0]), [41, 4], [1, 18]],
                        )
                        nc.vector.tensor_copy(dstA, ptA[:])
                    # pass A metadata: -> flat row idx (+ frac, kept for B)
                    for n2 in range(9):
                        a2 = n2 // 3
                        oyx = pst1.tile([128, HK, 2], F32, tag="sel")
                        for r in range(3):
                            ch = 3 * r + a2
                            rhs = bass.AP(
                                tensor=opm[:].tensor,
                                offset=opm[:].offset + ch + ho * 41,
                                ap=[list(opm[:].ap[0]), [41, HK], [9, 2]],
                            )
                            nc.tensor.matmul(oyx[:], selt[:, n2, r, :], rhs,
                                             start=(r == 0), stop=(r == 2))
                        P = meta.tile([128, HK, 2], F32, tag="P")
                        nc.vector.tensor_add(P[:], oyx[:],
                                             basyx[:, n2, ho:ho + HK, :])
                        nc.vector.tensor_scalar(P[:], P[:], 0.0, 63.0,
                                                AO.max, AO.min)
                        R0 = meta.tile([128, HK, 2], F32, tag="R0")
                        nc.vector.tensor_scalar(R0[:], P[:], -0.5, 12582912.0,
                                                AO.add, AO.add)
                        nc.vector.tensor_scalar_add(R0[:], R0[:], -12582912.0)
                        nc.vector.scalar_tensor_tensor(
                            idxt[:, n2, ho:ho + HK], R0[:, :, 1], 64.0,
                            R0[:, :, 0], AO.mult, AO.add)
                        # frac (for scales, pass B) computed off the idx path
                        F = Fall[:, n2, ho:ho + HK, :]
                        nc.vector.tensor_sub(F, P[:], R0[:])
                    # pass B: mod conv
                    for tl in range(tl_lo, tl_hi):
                        pm = psc.tile([9, 512], F32, tag="pm")
                        for t in range(9):
                            dy, dx = t // 3, t % 3
                            rhs2 = bass.AP(
                                tensor=xpad[:].tensor,
                                offset=xpad[:].offset + dx * 66 + dy + tl * 8,
                                ap=[list(xpad[:].ap[0]), [1, 8], [66, 64]],
                            )
                            nc.tensor.matmul(pm[:], wmtb[:, t, :], rhs2,
                                             start=(t == 0), stop=(t == 8))
                        nc.scalar.activation(
                            ocm[32:41, tl * 512:(tl + 1) * 512], pm[:],
                            mybir.ActivationFunctionType.Sigmoid,
                            bias=mb[:], scale=1.0)
                    for tl in range(tl_lo, tl_hi):
                        ptB = pst1.tile([128, 4, 9], F32, tag="ptB", bufs=1)
                        for k4 in range(4):
                            k = tl * 4 + k4
                            nc.tensor.matmul(
                                ptB[:, k4, :],
                                ocm[32:41, k * 128:(k + 1) * 128],
                                id16[32:41, 32:41], start=True, stop=True)
                        dstB = bass.AP(
                            tensor=opm[:].tensor,
                            offset=opm[:].offset + tl * 4 * 41 + 32,
                            ap=[list(opm[:].ap[0]), [41, 4], [1, 9]],
                        )
                        nc.vector.tensor_copy(dstB, ptB[:])
                    # pass B metadata: bilinear*modulation scales (fp16)
                    for n2 in range(9):
                        F = Fall[:, n2, ho:ho + HK, :]
                        mrow = opm[:, ho:ho + HK, 32 + n2]
                        v1 = meta.tile([128, HK], F32, tag="v1")
                        v0 = meta.tile([128, HK], F32, tag="v0")
                        sc4 = meta.tile([128, 4, HK], F32, tag="sc4")
                        nc.vector.tensor_mul(v1[:], mrow, F[:, :, 1])
                        nc.vector.tensor_sub(v0[:], mrow, v1[:])
                        nc.vector.tensor_mul(sc4[:, 1, :], v0[:], F[:, :, 0])
                        nc.vector.tensor_sub(sc4[:, 0, :], v0[:], sc4[:, 1, :])
                        nc.vector.tensor_mul(sc4[:, 3, :], v1[:], F[:, :, 0])
                        nc.vector.tensor_sub(sc4[:, 2, :], v1[:], sc4[:, 3, :])
                        # convert to fp16 [k, corner]-interleaved in one copy
                        csrc = bass.AP(
                            tensor=sc4[:].tensor, offset=sc4[:].offset,
                            ap=[list(sc4[:].ap[0]), [1, HK], [HK, 4]],
                        )
                        cdst = bass.AP(
                            tensor=scal[:].tensor,
                            offset=scal[:].offset + n2 * (KCH * 4) + ho * 4,
                            ap=[list(scal[:].ap[0]), [4, HK], [1, 4]],
                        )
                        nc.vector.tensor_copy(cdst, csrc)

            with (
                tc.tile_pool(name="pst", bufs=3, space="PSUM") as pst,
                tc.tile_pool(name="psm", bufs=1, space="PSUM") as psm,
            ):
                # ---- per spatial-quarter: gather + scale + reduce-transpose;
                #      main-conv matmuls interleave per n2 (PSUM accumulates
                #      while later n2 groups are still gathering)
                for sq in range(4):
                    vc = vbuf.tile([C, 9, 1024], F16, tag="vc")
                    accq = {}
                    for hf in range(2):
                        for tl2 in range(2):
                            accq[hf, tl2] = psm.tile(
                                [128, 512], F32, tag=f"mm{hf}{tl2}",
                                name=f"acc{hf}{tl2}")
                    for n2 in range(9):
                        g = gbuf.tile([128, 8, 512], F16, tag="g")
                        for kk in range(8):
                            k = sq * 8 + kk
                            dstg = bass.AP(
                                tensor=g[:].tensor,
                                offset=g[:].offset + kk * 512,
                                ap=[list(g[:].ap[0]), [1, 512]],
                            )
                            nc.gpsimd.indirect_dma_start(
                                out=dstg, out_offset=None,
                                in_=d["ptab"][:],
                                in_offset=bass.IndirectOffsetOnAxis(
                                    ap=idxt[:, n2, k:k + 1], axis=0),
                            )
                        h = hbuf.tile([128, 8, 128, 2], F16, tag="h")
                        # the very last group runs in 2-chunk units to halve
                        # the end-of-kernel dependency chain
                        last = (sq == 3 and n2 == 8)
                        units = ((0, 2), (2, 4), (4, 6), (6, 8)) if last \
                            else ((0, 4), (4, 8))
                        for klo, khi in units:
                            w = khi - klo
                            gv = bass.AP(
                                tensor=g[:].tensor,
                                offset=g[:].offset + klo * 512,
                                ap=[list(g[:].ap[0]), [512, w], [4, 128], [1, 4]],
                            )
                            sv = bass.AP(
                                tensor=scal[:].tensor,
                                offset=(scal[:].offset + n2 * (KCH * 4)
                                        + sq * 32 + klo * 4),
                                ap=[list(scal[:].ap[0]), [4, w], [0, 128], [1, 4]],
                            )
                            nc.vector.tensor_mul(gv, gv, sv)
                            ha = bass.AP(
                                tensor=g[:].tensor,
                                offset=g[:].offset + klo * 512,
                                ap=[list(g[:].ap[0]), [512, w], [4, 128], [1, 2]],
                            )
                            hb = bass.AP(
                                tensor=g[:].tensor,
                                offset=g[:].offset + klo * 512 + 2,
                                ap=[list(g[:].ap[0]), [512, w], [4, 128], [1, 2]],
                            )
                            hd = bass.AP(
                                tensor=h[:].tensor,
                                offset=h[:].offset + klo * 256,
                                ap=[list(h[:].ap[0]), [256, w], [2, 128], [1, 2]],
                            )
                            nc.vector.tensor_add(hd, ha, hb)
                            acc = pst.tile([128, w * 128], F32,
                                           tag="trs" if last else "tr",
                                           bufs=1 if last else None)
                            for kki in range(w):
                                kk = klo + kki
                                for j in range(2):
                                    lhsT = bass.AP(
                                        tensor=h[:].tensor,
                                        offset=h[:].offset + kk * 256 + j,
                                        ap=[list(h[:].ap[0]), [2, 128]],
                                    )
                                    nc.tensor.matmul(
                                        acc[:, kki * 128:(kki + 1) * 128],
                                        lhsT, id16[:],
                                        start=(j == 0), stop=(j == 1))
                            nc.scalar.copy(
                                vc[:, n2, klo * 128:khi * 128], acc[:])
                            q = klo // 4
                            co = (klo - q * 4) * 128
                            # close accq(1,*) first at the very end so its
                            # Act copy overlaps accq(0,*)'s final matmul
                            for hf in ((1, 0) if (last and klo == 6)
                                       else (0, 1)):
                                nc.tensor.matmul(
                                    accq[hf, q][:, co:co + w * 128],
                                    w2[:, n2, hf, :],
                                    vc[:, n2, klo * 128:khi * 128],
                                    start=(n2 == 0), stop=(n2 == 8),
                                    skip_group_check=last)

                    # store raw blocks (contiguous; host unscrambles); the two
                    # late q=1 copies run in parallel on Act and DVE
                    for hf in range(2):
                        for q in range(2):
                            outq = obuf.tile([128, 512], F32, tag="oq",
                                             name="outq")
                            if q == 1 and hf == 0:
                                nc.vector.tensor_copy(outq[:], accq[hf, q][:])
                            else:
                                nc.scalar.copy(outq[:], accq[hf, q][:])
                            nc.sync.dma_start(
                                d["outr"][sq * 4 + hf * 2 + q], outq[:])

    nc.compile()
    _CACHE["nc"] = nc
    return nc


def _host_inputs(b_x, offset_w, offset_b, mod_w, mod_b, conv_w):
    hc = _build_host_constants()
    img = b_x.astype(np.float32)
    womb = np.zeros((C, 9, 18), np.float16)
    wmtb = np.zeros((C, 9, 9), np.float16)
    for t in range(9):
        dy, dx = t // 3, t % 3
        womb[:, t, :] = offset_w[:, :, dy, dx].T
        wmtb[:, 3 * dx + dy, :] = mod_w[:, :, dy, dx].T
    w2 = np.zeros((C, 9, 2, 128), np.float16)
    for n2 in range(9):
        a2, e2 = n2 // 3, n2 % 3
        for hf in range(2):
            w2[:, n2, hf, :] = conv_w[128 * hf:128 * (hf + 1), :, a2, e2].T
    # fold the offset-conv bias into the base positions (it passes linearly
    # through the selection matmul; selected channel = chy (y) / chy+9 (x))
    ob32 = offset_b.astype(np.float32)
    basyx = hc["basyx"].copy()
    basyx[:, :, :, 0] += ob32[hc["chy"]][:, :, None]
    basyx[:, :, :, 1] += ob32[hc["chy"] + 9][:, :, None]
    xpad = _pad66(img)
    return {
        "xpad": xpad,
        "xw0": np.ascontiguousarray(
            np.concatenate([xpad[:, 0:660], womb.reshape(C, 162)], axis=1)),
        "ptab": _patch_table(img),
        "womb": womb,
        "wmtb": wmtb,
        "mb": mod_b.reshape(9, 1).astype(np.float32),
        "selt": hc["sel"].reshape(128, 9 * 3 * 128),
        "basyx": basyx.reshape(128, 9 * KCH * 2),
        "w2": w2.reshape(C, 9 * 2 * 128),
        "id16": hc["ident16"],
    }


def kernel(x, offset_w, offset_b, mod_w, mod_b, conv_w):
    nc = _build_program()
    in_maps = [
        _host_inputs(x[b], offset_w, offset_b, mod_w, mod_b, conv_w)
        for b in range(B)
    ]
    res = run_bass_kernel_spmd(nc, in_maps, core_ids=list(range(B)))
    out = np.empty((B, OUT, H, W), np.float32)
    for b in range(B):
        # outr[sq*4 + hf*2 + q] = [128 o, 512 pi2'] with
        # pi2' = (2sq+q)*512 + q2, j2 = 8*(2sq+q) + q2//64, i2 = q2%64
        outr = res.results[b]["outr"].reshape(4, 2, 2, 128, 8, 64)
        for sq in range(4):
            for hf in range(2):
                for q in range(2):
                    j2 = 16 * sq + 8 * q
                    out[b, 128 * hf:128 * (hf + 1), :, j2:j2 + 8] = (
                        outr[sq, hf, q].transpose(0, 2, 1))
    return out


if __name__ == "__main__":
    rng = np.random.default_rng(0)
    ins = {
        "x": rng.standard_normal((B, C, H, W), dtype=np.float32),
        "offset_w": (rng.standard_normal((18, C, 3, 3)) / 34).astype(np.float32),
        "offset_b": (rng.standard_normal(18) * 0.01).astype(np.float32),
        "mod_w": (rng.standard_normal((9, C, 3, 3)) / 34).astype(np.float32),
        "mod_b": (rng.standard_normal(9) * 0.01).astype(np.float32),
        "conv_w": (rng.standard_normal((OUT, C, 3, 3)) / 34).astype(np.float32),
    }
    o = kernel(**ins)
    print("out", o.shape, o.dtype, np.abs(o).max())
